# revision 44
# baseline (speedup 1.0000x reference)
"""Trainium2 Bass kernel for LittleBitLinearHF.

Computation (per reference):
    y = ((x * g) @ sign(V) * ell) @ sign(U).T * h + bias
with x (4, 2048, 4096) f32, U/V (4096, 128), rank r=128.

Strategy (memory-roofline oriented; tolerance is rel_err < 2e-2):
  * Data-parallel: 8192 tokens over 8 cores (1024 each), params replicated.
  * Quantization plan (host-side, measured end-to-end err 1.43e-2 on the
    deterministic seed-0 inputs):
      - xq  = e3m4(x * g * 2^-k[d])   1 byte/elt  (k per d_in column keeps
              |values| <= 15.5; k==0 for this data)
      - vs  = sign(V) * 2^k[d]        e3m4, EXACT (+-pow2)
      - uf  = ell * (sign(U)*h).T     bf16 (r, d_out)
      - y1 evacuated to bf16; output written bf16, upconverted on host.
    GEMM1 runs fp8e3 at 1x rate (full-precision upcast path keeps m4).
  * Layouts are fully host-packed so every DMA is contiguous per partition:
      xq  [p, c, dt, t]   chunks c of 512 tokens, dt = d_in/128 tile
      vs  [p, dt, r]
      y   [p, c, ot, t]   ot = d_out/128 tile (host unpacks + adds nothing)
  * Device per chunk c (512 tokens):
      GEMM1: y1(r=128, 512) += vs[:,dt,:].T @ xq[:,dt,:] over 32 dt (PSUM)
      y1 -> bf16 (gpsimd)
      GEMM2: out(o=128, 512) = uf[:,ot].T @ y1  per ot; evac adds per-
             partition bias (DVE tensor_scalar / ACT Identity+bias) -> bf16
      out groups of 8 ot DMA'd out on sync/scalar queues.
    Queues: x on tensor engine queue, params on gpsimd, out on sync+scalar.
"""

import ml_dtypes
import numpy as np

import concourse.bass as bass
import concourse.mybir as mybir
import concourse.tile as tile
from concourse.bass_utils import run_bass_kernel_spmd

N_CORES = 8
B, S, D_IN, D_OUT, R = 4, 2048, 4096, 4096, 128
T = B * S                      # 8192 tokens
T_CORE = T // N_CORES          # 1024 tokens per core
T_CHUNK = 512                  # tokens per chunk (one PSUM bank of f32)
N_CHUNKS = T_CORE // T_CHUNK
P = 128
N_DT = D_IN // P               # 32 d_in tiles
N_OT = D_OUT // P              # 32 d_out tiles
X_GRP = 16                     # dt tiles per x sub-DMA (1 MiB, 8 KiB packets)
O_GRP = 8                      # ot tiles per out sub-DMA (1 MiB, 8 KiB packets)
F32 = mybir.dt.float32
BF16 = mybir.dt.bfloat16
FP8 = mybir.dt.float8e3

USE_FP8 = True                 # False -> bf16 x (safer, ~4.4e-3 err)
XDT = FP8 if USE_FP8 else BF16
E3M4_MAX = 15.5

_CACHED = {}

# evac engine per ot: V=vector (594ns/tile), A=scalar ACT (720ns/tile);
# 9:7 split balances the two lanes. (gpsimd cannot access PSUM)
_EVAC_PATTERN = "VAVAVAVAVAVAVVAV"


def _build_nc():
    from concourse.bacc import Bacc
    nc = Bacc()
    xq = nc.dram_tensor("xq", [P, N_CHUNKS * N_DT * T_CHUNK], XDT,
                        kind="ExternalInput")
    vs = nc.dram_tensor("vs", [P, N_DT * R], XDT, kind="ExternalInput")
    uf = nc.dram_tensor("uf", [R, D_OUT], BF16, kind="ExternalInput")
    bp = nc.dram_tensor("bp", [P, N_OT], F32, kind="ExternalInput")
    y = nc.dram_tensor("y", [P, N_CHUNKS * N_OT * T_CHUNK], BF16,
                       kind="ExternalOutput")

    with tile.TileContext(nc) as tc:
        with (
            tc.tile_pool(name="params", bufs=1) as ppool,
            tc.tile_pool(name="xin", bufs=2) as xpool,
            tc.tile_pool(name="y1sb", bufs=2) as y1pool,
            tc.tile_pool(name="outsb", bufs=2) as opool,
            tc.tile_pool(name="ps_y1", bufs=1, space=bass.MemorySpace.PSUM) as ps1,
            tc.tile_pool(name="ps_o", bufs=5, space=bass.MemorySpace.PSUM) as ps2,
        ):
            # ---- params: vs halves race on sync/gpsimd ahead of x; bp on
            # scalar (uf only needed by GEMM2, loaded mid-stream) ---------
            vs_sb = ppool.tile([P, N_DT, R], XDT)
            nc.sync.dma_start(
                vs_sb[:, 0:N_DT // 2, :],
                vs[:, 0:N_DT // 2 * R].rearrange("p (n r) -> p n r",
                                                 n=N_DT // 2))
            nc.gpsimd.dma_start(
                vs_sb[:, N_DT // 2:, :],
                vs[:, N_DT // 2 * R:].rearrange("p (n r) -> p n r",
                                                n=N_DT // 2))
            bp_sb = ppool.tile([P, N_OT], F32)
            nc.scalar.dma_start(bp_sb[:], bp[:])

            # ---- x sub-DMAs: 8-dt pieces round-robin over all 3 queues,
            # accumulation order == arrival order so GEMM1 streams; uf is
            # split in half and raced on gpsimd/scalar between chunks ----
            x_sb = []
            uf_sb = ppool.tile([R, D_OUT], BF16)
            # chunk0's first piece is tiny (2 dt) so GEMM1 issues early and
            # the PE's HAM cold-start penalty hides behind the read phase
            x_splits = {0: [(0, 2, nc.sync), (2, 8, nc.sync),
                            (8, 16, nc.gpsimd),
                            (16, 24, nc.scalar), (24, 32, nc.sync)],
                        1: [(0, 8, nc.sync), (8, 16, nc.gpsimd),
                            (16, 24, nc.scalar), (24, 32, nc.gpsimd)]}

            def load_x(c):
                xt = xpool.tile([P, N_DT * T_CHUNK], XDT, tag="x", name=f"x{c}")
                x_sb.append(xt)
                for dt0, dt1, xdq in x_splits[c]:
                    lo = c * N_DT * T_CHUNK + dt0 * T_CHUNK
                    hi = c * N_DT * T_CHUNK + dt1 * T_CHUNK
                    xdq.dma_start(
                        xt[:, dt0 * T_CHUNK:dt1 * T_CHUNK],
                        xq[:, lo:hi])

            load_x(0)
            nc.gpsimd.dma_start(uf_sb[:, 0:D_OUT // 2], uf[:, 0:D_OUT // 2])
            nc.scalar.dma_start(uf_sb[:, D_OUT // 2:], uf[:, D_OUT // 2:])
            load_x(1)

            y1_sbs = [None, None]
            out_sbs = [None, None]

            def gemm1(c, dt0, dt1):
                for dt in range(dt0, dt1):
                    nc.tensor.matmul(
                        _g1ps[c][:],
                        vs_sb[:, dt, :],
                        x_sb[c][:, dt * T_CHUNK:(dt + 1) * T_CHUNK],
                        start=(dt == 0),
                        stop=(dt == N_DT - 1),
                    )

            def y1_evac(c):
                y1_sb = y1pool.tile([R, T_CHUNK], BF16, name=f"y1sb{c}")
                nc.vector.tensor_copy(y1_sb[:], _g1ps[c][:])
                return y1_sb

            def gemm2(c, y1_sb, out_sb, ot0, ot1):
                for ot in range(ot0, ot1):
                    ps = ps2.tile([P, T_CHUNK], F32)
                    nc.tensor.matmul(
                        ps[:],
                        uf_sb[:, ot * P:(ot + 1) * P],
                        y1_sb[:],
                        start=True,
                        stop=True,
                    )
                    osl = out_sb[:, ot * T_CHUNK:(ot + 1) * T_CHUNK]
                    ev = _EVAC_PATTERN[ot % len(_EVAC_PATTERN)]
                    if ev == "A":
                        nc.scalar.activation(
                            osl, ps[:],
                            mybir.ActivationFunctionType.Identity,
                            bias=bp_sb[:, ot:ot + 1])
                    else:
                        nc.vector.tensor_scalar_add(
                            osl, ps[:], bp_sb[:, ot:ot + 1])
                    if ot % O_GRP == O_GRP - 1:
                        g0 = ot - (O_GRP - 1)
                        gidx = c * (N_OT // O_GRP) + ot // O_GRP
                        if gidx == 7:
                            # split the final group across two idle queues
                            # to halve the drain tail
                            for h0, h1, dq in ((g0, g0 + 4, nc.sync),
                                               (g0 + 4, ot + 1, nc.scalar)):
                                lo = c * N_OT * T_CHUNK + h0 * T_CHUNK
                                dq.dma_start(
                                    y[:, lo:lo + (h1 - h0) * T_CHUNK],
                                    out_sb[:, h0 * T_CHUNK:h1 * T_CHUNK])
                        else:
                            lo = c * N_OT * T_CHUNK + g0 * T_CHUNK
                            hi = lo + O_GRP * T_CHUNK
                            dq = (nc.scalar, nc.gpsimd, nc.sync)[gidx % 3]
                            dq.dma_start(
                                y[:, lo:hi],
                                out_sb[:, g0 * T_CHUNK:(ot + 1) * T_CHUNK])

            # Tensor stream: serial chunks. (Both a fine-grained interleave
            # and splitting G2c0 around G1c1 measured slower -- the evac
            # lanes pace G2 either way, and moving PE work earlier delays
            # c0's write production.)
            _g1ps = [ps1.tile([R, T_CHUNK], F32, name=f"y1ps{c}")
                     for c in range(N_CHUNKS)]
            out_sbs = [opool.tile([P, N_OT * T_CHUNK], BF16, name=f"osb{c}")
                       for c in range(N_CHUNKS)]
            for c in range(N_CHUNKS):
                gemm1(c, 0, N_DT)
                y1_c = y1_evac(c)
                gemm2(c, y1_c, out_sbs[c], 0, N_OT)

    nc.finalize()
    return nc


def _get_nc():
    if "nc" not in _CACHED:
        _CACHED["nc"] = _build_nc()
    return _CACHED["nc"]


def _prep_inputs(x, U_fp, V_fp, h, g, ell, bias):
    x = np.asarray(x, dtype=np.float32).reshape(T, D_IN)
    U_fp = np.asarray(U_fp, dtype=np.float32)
    V_fp = np.asarray(V_fp, dtype=np.float32)
    h = np.asarray(h, dtype=np.float32)
    g = np.asarray(g, dtype=np.float32)
    ell = np.asarray(ell, dtype=np.float32)
    bias = np.asarray(bias, dtype=np.float32)

    U_sign = np.where(U_fp >= 0, np.float32(1.0), np.float32(-1.0))
    V_sign = np.where(V_fp >= 0, np.float32(1.0), np.float32(-1.0))

    np_xdt = mybir.dt.np(XDT)
    if USE_FP8:
        xg = x * g[None, :]
        # per-column power-of-two scale so |xq| <= 15.5 (exact inverse on vs)
        mx = np.abs(xg).max(axis=0)
        k = np.maximum(0, np.ceil(np.log2(np.maximum(mx, 1e-30) / E3M4_MAX)))
        k = k.astype(np.float32)
        assert k.max() <= 3.0, "pow2 scale exceeds e3m4 range"
        scale = (2.0 ** k).astype(np.float32)
        xh = np.clip(xg / scale[None, :], -E3M4_MAX, E3M4_MAX).astype(np_xdt)
        vs_host = (V_sign * scale[:, None]).astype(np_xdt)
    else:
        xh = x.astype(np_xdt)
        vs_host = (V_sign * g[:, None]).astype(np_xdt)

    # pack vs (d_in, r) -> (p, dt*r)
    vs_host = np.ascontiguousarray(
        vs_host.reshape(N_DT, P, R).transpose(1, 0, 2).reshape(P, N_DT * R))
    uf_host = np.ascontiguousarray(
        (ell[:, None] * (U_sign * h[:, None]).T).astype(ml_dtypes.bfloat16))
    bp_host = np.ascontiguousarray(bias.reshape(N_OT, P).T)

    in_maps = []
    for cidx in range(N_CORES):
        shard = xh[cidx * T_CORE:(cidx + 1) * T_CORE]      # (1024, 4096)
        xp = shard.reshape(N_CHUNKS, T_CHUNK, N_DT, P)
        xp = np.ascontiguousarray(
            xp.transpose(3, 0, 2, 1).reshape(P, N_CHUNKS * N_DT * T_CHUNK))
        in_maps.append({
            "xq": xp,
            "vs": vs_host,
            "uf": uf_host,
            "bp": bp_host,
        })
    return in_maps


def _unpack_core(yp):
    """(P, N_CHUNKS*N_OT*T_CHUNK) packed bf16 -> (T_CORE, D_OUT) f32."""
    yp = np.asarray(yp).reshape(P, N_CHUNKS, N_OT, T_CHUNK)
    return yp.transpose(1, 3, 2, 0).reshape(T_CORE, D_OUT).astype(np.float32)


def _unpack_output(res):
    outs = [_unpack_core(res.results[c]["y"]) for c in range(N_CORES)]
    return np.concatenate(outs, axis=0).reshape(B, S, D_OUT)


def kernel(x, U_fp, V_fp, h, g, ell, bias, _run_kwargs=None):
    in_maps = _prep_inputs(x, U_fp, V_fp, h, g, ell, bias)
    nc = _get_nc()
    kw = _run_kwargs or {}
    res = run_bass_kernel_spmd(nc, in_maps, list(range(N_CORES)), **kw)
    if _run_kwargs is not None:
        _CACHED["last_results"] = res
    return _unpack_output(res)


# revision 45
# speedup vs baseline: 1.1186x; 1.1186x over previous
"""Trainium2 Bass kernel for LittleBitLinearHF.

Computation (per reference):
    y = ((x * g) @ sign(V) * ell) @ sign(U).T * h + bias
with x (4, 2048, 4096) f32, U/V (4096, 128), rank r=128.

Strategy (memory-roofline oriented; tolerance is rel_err < 2e-2):
  * Data-parallel: 8192 tokens over 8 cores (1024 each), params replicated.
  * Quantization plan (host-side, measured end-to-end err 1.43e-2 on the
    deterministic seed-0 inputs):
      - xq  = e3m4(x * g * 2^-k[d])   1 byte/elt  (k per d_in column keeps
              |values| <= 15.5; k==0 for this data)
      - vs  = sign(V) * 2^k[d]        e3m4, EXACT (+-pow2)
      - uf  = ell * (sign(U)*h).T     bf16 (r, d_out)
      - y1 evacuated to bf16; output written bf16, upconverted on host.
    GEMM1 runs fp8e3 at 1x rate (full-precision upcast path keeps m4).
  * Layouts are fully host-packed so every DMA is contiguous per partition:
      xq  [p, c, dt, t]   chunks c of 512 tokens, dt = d_in/128 tile
      vs  [p, dt, r]
      y   [p, c, ot, t]   ot = d_out/128 tile (host unpacks + adds nothing)
  * Device per chunk c (512 tokens):
      GEMM1: y1(r=128, 512) += vs[:,dt,:].T @ xq[:,dt,:] over 32 dt (PSUM)
      y1 -> bf16 (gpsimd)
      GEMM2: out(o=128, 512) = uf[:,ot].T @ y1  per ot; evac adds per-
             partition bias (DVE tensor_scalar / ACT Identity+bias) -> bf16
      out groups of 8 ot DMA'd out on sync/scalar queues.
    Queues: x on tensor engine queue, params on gpsimd, out on sync+scalar.
"""

import ml_dtypes
import numpy as np

import concourse.bass as bass
import concourse.mybir as mybir
import concourse.tile as tile
from concourse.bass_utils import run_bass_kernel_spmd

N_CORES = 8
B, S, D_IN, D_OUT, R = 4, 2048, 4096, 4096, 128
T = B * S                      # 8192 tokens
T_CORE = T // N_CORES          # 1024 tokens per core
T_CHUNK = 512                  # tokens per chunk (one PSUM bank of f32)
N_CHUNKS = T_CORE // T_CHUNK
P = 128
N_DT = D_IN // P               # 32 d_in tiles
N_OT = D_OUT // P              # 32 d_out tiles
X_GRP = 16                     # dt tiles per x sub-DMA (1 MiB, 8 KiB packets)
O_GRP = 8                      # ot tiles per out sub-DMA (1 MiB, 8 KiB packets)
F32 = mybir.dt.float32
BF16 = mybir.dt.bfloat16
FP8 = mybir.dt.float8e3

USE_FP8 = True                 # False -> bf16 x (safer, ~4.4e-3 err)
XDT = FP8 if USE_FP8 else BF16
E3M4_MAX = 15.5

_CACHED = {}

# evac engine per ot: V=vector (594ns/tile), A=scalar ACT (720ns/tile);
# 9:7 split balances the two lanes. (gpsimd cannot access PSUM)
_EVAC_PATTERN = "VAVAVAVAVAVAVVAV"


def _build_nc():
    from concourse.bacc import Bacc
    nc = Bacc()
    xq = nc.dram_tensor("xq", [P, N_CHUNKS * N_DT * T_CHUNK], XDT,
                        kind="ExternalInput")
    vs = nc.dram_tensor("vs", [P, N_DT * R], XDT, kind="ExternalInput")
    uf = nc.dram_tensor("uf", [R, D_OUT], BF16, kind="ExternalInput")
    bp = nc.dram_tensor("bp", [P, N_OT], F32, kind="ExternalInput")
    y = nc.dram_tensor("y", [P, N_CHUNKS * N_OT * T_CHUNK], BF16,
                       kind="ExternalOutput")

    with tile.TileContext(nc) as tc:
        with (
            tc.tile_pool(name="params", bufs=1) as ppool,
            tc.tile_pool(name="xin", bufs=2) as xpool,
            tc.tile_pool(name="y1sb", bufs=2) as y1pool,
            tc.tile_pool(name="outsb", bufs=2) as opool,
            tc.tile_pool(name="ps_y1", bufs=1, space=bass.MemorySpace.PSUM) as ps1,
            tc.tile_pool(name="ps_o", bufs=5, space=bass.MemorySpace.PSUM) as ps2,
        ):
            # ---- params: vs halves race on sync/gpsimd ahead of x; bp on
            # scalar (uf only needed by GEMM2, loaded mid-stream) ---------
            vs_sb = ppool.tile([P, N_DT, R], XDT)
            nc.sync.dma_start(
                vs_sb[:, 0:N_DT // 2, :],
                vs[:, 0:N_DT // 2 * R].rearrange("p (n r) -> p n r",
                                                 n=N_DT // 2))
            nc.gpsimd.dma_start(
                vs_sb[:, N_DT // 2:, :],
                vs[:, N_DT // 2 * R:].rearrange("p (n r) -> p n r",
                                                n=N_DT // 2))
            bp_sb = ppool.tile([P, N_OT], F32)
            nc.scalar.dma_start(bp_sb[:], bp[:])

            # ---- x sub-DMAs: 8-dt pieces round-robin over all 3 queues,
            # accumulation order == arrival order so GEMM1 streams; uf is
            # split in half and raced on gpsimd/scalar between chunks ----
            x_sb = []
            uf_sb = ppool.tile([R, D_OUT], BF16)
            x_splits = {0: [(0, 8, nc.sync), (8, 16, nc.gpsimd),
                            (16, 24, nc.scalar), (24, 32, nc.sync)],
                        1: [(0, 8, nc.sync), (8, 16, nc.gpsimd),
                            (16, 24, nc.scalar), (24, 32, nc.gpsimd)]}

            def load_x(c):
                xt = xpool.tile([P, N_DT * T_CHUNK], XDT, tag="x", name=f"x{c}")
                x_sb.append(xt)
                for dt0, dt1, xdq in x_splits[c]:
                    lo = c * N_DT * T_CHUNK + dt0 * T_CHUNK
                    hi = c * N_DT * T_CHUNK + dt1 * T_CHUNK
                    xdq.dma_start(
                        xt[:, dt0 * T_CHUNK:dt1 * T_CHUNK],
                        xq[:, lo:hi])

            load_x(0)
            nc.gpsimd.dma_start(uf_sb[:, 0:D_OUT // 2], uf[:, 0:D_OUT // 2])
            nc.scalar.dma_start(uf_sb[:, D_OUT // 2:], uf[:, D_OUT // 2:])
            load_x(1)

            y1_sbs = [None, None]
            out_sbs = [None, None]

            def gemm1(c, dt0, dt1):
                for dt in range(dt0, dt1):
                    nc.tensor.matmul(
                        _g1ps[c][:],
                        vs_sb[:, dt, :],
                        x_sb[c][:, dt * T_CHUNK:(dt + 1) * T_CHUNK],
                        start=(dt == 0),
                        stop=(dt == N_DT - 1),
                    )

            def y1_evac(c):
                y1_sb = y1pool.tile([R, T_CHUNK], BF16, name=f"y1sb{c}")
                nc.vector.tensor_copy(y1_sb[:], _g1ps[c][:])
                return y1_sb

            def gemm2(c, y1_sb, out_sb, ot0, ot1):
                for ot in range(ot0, ot1):
                    ps = ps2.tile([P, T_CHUNK], F32)
                    nc.tensor.matmul(
                        ps[:],
                        uf_sb[:, ot * P:(ot + 1) * P],
                        y1_sb[:],
                        start=True,
                        stop=True,
                    )
                    osl = out_sb[:, ot * T_CHUNK:(ot + 1) * T_CHUNK]
                    ev = _EVAC_PATTERN[ot % len(_EVAC_PATTERN)]
                    if ev == "A":
                        nc.scalar.activation(
                            osl, ps[:],
                            mybir.ActivationFunctionType.Identity,
                            bias=bp_sb[:, ot:ot + 1])
                    else:
                        nc.vector.tensor_scalar_add(
                            osl, ps[:], bp_sb[:, ot:ot + 1])
                    if ot % O_GRP == O_GRP - 1:
                        g0 = ot - (O_GRP - 1)
                        gidx = c * (N_OT // O_GRP) + ot // O_GRP
                        if gidx == 7:
                            # split the final group across two idle queues
                            # to halve the drain tail
                            for h0, h1, dq in ((g0, g0 + 4, nc.sync),
                                               (g0 + 4, ot + 1, nc.scalar)):
                                lo = c * N_OT * T_CHUNK + h0 * T_CHUNK
                                dq.dma_start(
                                    y[:, lo:lo + (h1 - h0) * T_CHUNK],
                                    out_sb[:, h0 * T_CHUNK:h1 * T_CHUNK])
                        else:
                            lo = c * N_OT * T_CHUNK + g0 * T_CHUNK
                            hi = lo + O_GRP * T_CHUNK
                            dq = (nc.scalar, nc.gpsimd, nc.sync)[gidx % 3]
                            dq.dma_start(
                                y[:, lo:hi],
                                out_sb[:, g0 * T_CHUNK:(ot + 1) * T_CHUNK])

            # Tensor stream: serial chunks. (Both a fine-grained interleave
            # and splitting G2c0 around G1c1 measured slower -- the evac
            # lanes pace G2 either way, and moving PE work earlier delays
            # c0's write production.)
            _g1ps = [ps1.tile([R, T_CHUNK], F32, name=f"y1ps{c}")
                     for c in range(N_CHUNKS)]
            out_sbs = [opool.tile([P, N_OT * T_CHUNK], BF16, name=f"osb{c}")
                       for c in range(N_CHUNKS)]
            for c in range(N_CHUNKS):
                gemm1(c, 0, N_DT)
                y1_c = y1_evac(c)
                gemm2(c, y1_c, out_sbs[c], 0, N_OT)

    nc.finalize()
    return nc


def _get_nc():
    if "nc" not in _CACHED:
        _CACHED["nc"] = _build_nc()
    return _CACHED["nc"]


def _prep_inputs(x, U_fp, V_fp, h, g, ell, bias):
    x = np.asarray(x, dtype=np.float32).reshape(T, D_IN)
    U_fp = np.asarray(U_fp, dtype=np.float32)
    V_fp = np.asarray(V_fp, dtype=np.float32)
    h = np.asarray(h, dtype=np.float32)
    g = np.asarray(g, dtype=np.float32)
    ell = np.asarray(ell, dtype=np.float32)
    bias = np.asarray(bias, dtype=np.float32)

    U_sign = np.where(U_fp >= 0, np.float32(1.0), np.float32(-1.0))
    V_sign = np.where(V_fp >= 0, np.float32(1.0), np.float32(-1.0))

    np_xdt = mybir.dt.np(XDT)
    if USE_FP8:
        xg = x * g[None, :]
        # per-column power-of-two scale so |xq| <= 15.5 (exact inverse on vs)
        mx = np.abs(xg).max(axis=0)
        k = np.maximum(0, np.ceil(np.log2(np.maximum(mx, 1e-30) / E3M4_MAX)))
        k = k.astype(np.float32)
        assert k.max() <= 3.0, "pow2 scale exceeds e3m4 range"
        scale = (2.0 ** k).astype(np.float32)
        xh = np.clip(xg / scale[None, :], -E3M4_MAX, E3M4_MAX).astype(np_xdt)
        vs_host = (V_sign * scale[:, None]).astype(np_xdt)
    else:
        xh = x.astype(np_xdt)
        vs_host = (V_sign * g[:, None]).astype(np_xdt)

    # pack vs (d_in, r) -> (p, dt*r)
    vs_host = np.ascontiguousarray(
        vs_host.reshape(N_DT, P, R).transpose(1, 0, 2).reshape(P, N_DT * R))
    uf_host = np.ascontiguousarray(
        (ell[:, None] * (U_sign * h[:, None]).T).astype(ml_dtypes.bfloat16))
    bp_host = np.ascontiguousarray(bias.reshape(N_OT, P).T)

    in_maps = []
    for cidx in range(N_CORES):
        shard = xh[cidx * T_CORE:(cidx + 1) * T_CORE]      # (1024, 4096)
        xp = shard.reshape(N_CHUNKS, T_CHUNK, N_DT, P)
        xp = np.ascontiguousarray(
            xp.transpose(3, 0, 2, 1).reshape(P, N_CHUNKS * N_DT * T_CHUNK))
        in_maps.append({
            "xq": xp,
            "vs": vs_host,
            "uf": uf_host,
            "bp": bp_host,
        })
    return in_maps


def _unpack_core(yp):
    """(P, N_CHUNKS*N_OT*T_CHUNK) packed bf16 -> (T_CORE, D_OUT) f32."""
    yp = np.asarray(yp).reshape(P, N_CHUNKS, N_OT, T_CHUNK)
    return yp.transpose(1, 3, 2, 0).reshape(T_CORE, D_OUT).astype(np.float32)


def _unpack_output(res):
    outs = [_unpack_core(res.results[c]["y"]) for c in range(N_CORES)]
    return np.concatenate(outs, axis=0).reshape(B, S, D_OUT)


def kernel(x, U_fp, V_fp, h, g, ell, bias, _run_kwargs=None):
    in_maps = _prep_inputs(x, U_fp, V_fp, h, g, ell, bias)
    nc = _get_nc()
    kw = _run_kwargs or {}
    res = run_bass_kernel_spmd(nc, in_maps, list(range(N_CORES)), **kw)
    if _run_kwargs is not None:
        _CACHED["last_results"] = res
    return _unpack_output(res)
